# revision 25
# baseline (speedup 1.0000x reference)
"""ContrastiveDist kernel for TRN2 (8 NeuronCores, SPMD) -- v3.

out[n] = sum_e -(t_e . v_n) / (||t_e|| * ||v_n|| + eps)
       = -(s . v_n) / ||v_n||          with s = sum_e t_e / ||t_e||
(eps shifts the result by ~4e-11 relative -- dropped.)

Design (see trace notes in git history / transcript):
 * All inputs ship as bf16 (host cast): halves HBM traffic; 2e-2 budget
   has ~6x margin over measured 3e-3 bf16 noise.
 * node_emb ships TRANSPOSED ([256 d, 6272 n], d on partitions): the PE
   does both per-node reductions (dots via lhsT = s column, ssq via
   lhsT = ones column against squared v).  A block-diagonal lhsT built
   from an on-device identity routes group g's [1, 448] result to psum
   row g (PE out must start at partition 0/32/64, so partition-offset
   writes are not an option).
 * Groups split into two psum pairs (rows 0-6 / 7-13): the pair-A tail
   (sqrt + reciprocal + mul) runs while pair-B chunks still stream.
 * DVE squares v (bf16 2x packed TT); target ssq is split ACT
   (square+accum_out) / DVE (fused tensor_tensor_reduce); ACT keeps the
   sqrt_and_others table set (square is filler in every set) so there
   are no mid-kernel ACT_TABLE_LOADs after the t=0 preload.
 * One DMA ring (SP), FIFO: target (2 chunks) then node chunks
   [3,4,4,2,1]x448 cols -- s is ready before the first node chunk
   lands, the small last chunk shortens the drain.
 * PE prewarm matmuls keep the HAM clock gate busy so real matmuls run
   at 2.4GHz.
"""

import numpy as np
import ml_dtypes
from contextlib import ExitStack

import concourse.bacc as bacc
import concourse.bass as bass
import concourse.mybir as mybir
import concourse.tile as tile
from concourse import bass_utils

E, D = 2048, 256          # entities, embed dim
N_FULL = 50000            # total nodes
N_CORES = 8
NPC = N_FULL // N_CORES   # 6250 true nodes per core
G = 448                   # node columns per psum group (fp32 bank width)
NG = 14                   # groups per core -> NPAD = 6272
NPAD = G * NG
A = 2                     # d-halves (256 = 2*128 partitions)
ET = E // 128             # 16 entity tiles
TCH = 2                   # target DMA chunks
TSQ_ACT = 3               # per target chunk: this many tiles square on ACT
TSQ_MODE = "batched"      # "fused" (ACT accum + DVE TTR) or "batched"
VCH = [3, 4, 4, 2, 1]     # node chunk sizes in groups (sum = NG)
PAIR_SPLIT = 7            # groups 0..6 -> psum pair A, 7..13 -> pair B
SQ_ENG = "VVVVV"          # per-chunk square engine: V=vector, S=scalar
WARM_MM = 8               # PE prewarm dummy matmuls
TAIL_DIVIDE = False       # TT divide fails ISA check; reciprocal+mul

F32 = mybir.dt.float32
BF16 = mybir.dt.bfloat16
I16 = mybir.dt.int16
BF = ml_dtypes.bfloat16

_cache = {}


def _build():
    nc = bacc.Bacc(
        "TRN2",
        target_bir_lowering=False,
        debug=False,
        enable_asserts=True,
        num_devices=N_CORES,
    )
    tgt = nc.dram_tensor("target", [E, D], BF16, kind="ExternalInput").ap()
    vt = nc.dram_tensor("vt", [D, NPAD], BF16, kind="ExternalInput").ap()
    eye = nc.dram_tensor(
        "eye", [128, PAIR_SPLIT * PAIR_SPLIT], BF16, kind="ExternalInput"
    ).ap()
    out = nc.dram_tensor("out", [NPAD], F32, kind="ExternalOutput").ap()

    with tile.TileContext(nc) as tc, ExitStack() as ctx:
        tpool = ctx.enter_context(tc.tile_pool(name="tgt", bufs=1))
        vpool = ctx.enter_context(tc.tile_pool(name="v", bufs=1))
        spool = ctx.enter_context(tc.tile_pool(name="small", bufs=1))
        scr = ctx.enter_context(tc.tile_pool(name="scr", bufs=1))
        ps_w = ctx.enter_context(tc.tile_pool(name="psw", bufs=1, space="PSUM"))
        ps_s0 = ctx.enter_context(tc.tile_pool(name="pss0", bufs=1, space="PSUM"))
        ps_s1 = ctx.enter_context(tc.tile_pool(name="pss1", bufs=1, space="PSUM"))
        ps_da = ctx.enter_context(tc.tile_pool(name="psda", bufs=1, space="PSUM"))
        ps_qa = ctx.enter_context(tc.tile_pool(name="psqa", bufs=1, space="PSUM"))
        ps_db = ctx.enter_context(tc.tile_pool(name="psdb", bufs=1, space="PSUM"))
        ps_qb = ctx.enter_context(tc.tile_pool(name="psqb", bufs=1, space="PSUM"))

        NP = PAIR_SPLIT            # rows per pair (7)
        tgt_sb = tpool.tile([128, ET, D], BF16)
        tsq = scr.tile([128, ET, D], BF16, tag="tsq")
        vt_sb = vpool.tile([128, A, NPAD], BF16, tag="vt")
        vsq = vpool.tile([128, A, NPAD], BF16, tag="vsq")

        ssq_t = spool.tile([128, ET], F32, tag="ssqt")
        tn = spool.tile([128, ET], F32, tag="tn")
        inv_t = spool.tile([128, ET], F32, tag="invt")
        inv_bf = spool.tile([128, ET], BF16, tag="invbf")
        eye_sb = spool.tile([128, NP, NP], BF16, tag="eye")
        s_bf = spool.tile([128, A], BF16, tag="sbf")
        dotw = spool.tile([128, A, NP, NP], BF16, tag="dotw")
        warm_w = spool.tile([128, 1], BF16, tag="warmw")
        warm_x = spool.tile([128, G], BF16, tag="warmx")
        act_d = spool.tile([1, 1], F32, tag="actd")
        act_s = spool.tile([1, 1], F32, tag="acts")
        vn = [
            spool.tile([NP, G], F32, name="vna"),
            spool.tile([NP, G], F32, name="vnb"),
        ]
        isv = [
            spool.tile([NP, G], F32, name="isva"),
            spool.tile([NP, G], F32, name="isvb"),
        ]
        res = [
            spool.tile([NP, G], F32, name="resa"),
            spool.tile([NP, G], F32, name="resb"),
        ]

        warm_ps = ps_w.tile([1, G], F32)
        s_ps = [
            ps_s0.tile([128, 1], F32, name="sps0"),
            ps_s1.tile([128, 1], F32, name="sps1"),
        ]
        dot_ps = [
            ps_da.tile([NP, G], F32, name="dotpsa"),
            ps_db.tile([NP, G], F32, name="dotpsb"),
        ]
        sq_ps = [
            ps_qa.tile([NP, G], F32, name="sqpsa"),
            ps_qb.tile([NP, G], F32, name="sqpsb"),
        ]

        tgt_v = tgt.rearrange("(p j) d -> p j d", j=ET)
        vt_v = vt.rearrange("(a p) n -> p a n", p=128)
        out_v = out.rearrange("(g f) -> g f", f=G)

        # ---- DMAs (single SP HWDGE ring, FIFO)
        nc.sync.dma_start(eye_sb[:], eye.rearrange("p (g m) -> p g m", m=NP))
        H = ET // TCH
        for k in range(TCH):
            nc.sync.dma_start(
                tgt_sb[:, k * H : (k + 1) * H, :], tgt_v[:, k * H : (k + 1) * H, :]
            )
        chunks = []
        gb = 0
        for w in VCH:
            c0, c1 = gb * G, (gb + w) * G
            nc.sync.dma_start(vt_sb[:, :, c0:c1], vt_v[:, :, c0:c1])
            chunks.append((gb, w))
            gb += w

        # ---- consts
        nc.vector.memset(warm_w[:], 1.0)
        nc.vector.memset(warm_x[:], 0.0)
        nc.vector.memset(act_d[:], 1.0)

        # ---- ACT table preload (sqrt_and_others: sqrt + square filler)
        nc.scalar.sqrt(act_s[:], act_d[:])

        # ---- PE prewarm (HAM clock gate wants ~3.4us of activity)
        for _ in range(WARM_MM):
            nc.tensor.matmul(warm_ps[:], warm_w[:], warm_x[:], start=True, stop=True)

        # ---- phase A: s_a = -sum_e t_e/||t_e|| as lhsT columns [128, 1]
        for k in range(TCH):
            j0 = k * H
            sl = slice(j0, j0 + H)
            if TSQ_MODE == "fused":
                # first TSQ_ACT tiles on ACT (square+accum), rest fused
                # on DVE (tensor_tensor_reduce)
                for j in range(j0, j0 + TSQ_ACT):
                    nc.scalar.activation(
                        tsq[:, j, :], tgt_sb[:, j, :],
                        mybir.ActivationFunctionType.Square,
                        accum_out=ssq_t[:, j : j + 1],
                    )
                for j in range(j0 + TSQ_ACT, j0 + H):
                    nc.vector.tensor_tensor_reduce(
                        out=tsq[:, j, :],
                        in0=tgt_sb[:, j, :],
                        in1=tgt_sb[:, j, :],
                        scale=1.0,
                        scalar=0.0,
                        op0=mybir.AluOpType.mult,
                        op1=mybir.AluOpType.add,
                        accum_out=ssq_t[:, j : j + 1],
                    )
            else:
                nc.vector.tensor_mul(
                    tsq[:, sl, :], tgt_sb[:, sl, :], tgt_sb[:, sl, :]
                )
                nc.vector.tensor_reduce(
                    ssq_t[:, sl], tsq[:, sl, :],
                    axis=mybir.AxisListType.X, op=mybir.AluOpType.add,
                )
            nc.scalar.sqrt(tn[:, sl], ssq_t[:, sl])
            nc.vector.reciprocal(inv_t[:, sl], tn[:, sl])
            nc.vector.tensor_scalar_mul(inv_bf[:, sl], inv_t[:, sl], -1.0)
            for j in range(j0, j0 + H):
                for a in range(A):
                    nc.tensor.matmul(
                        s_ps[a][:],
                        tgt_sb[:, j, a * 128 : (a + 1) * 128],
                        inv_bf[:, j : j + 1],
                        start=(j == 0),
                        stop=(j == ET - 1),
                    )
        for a in range(A):
            nc.vector.tensor_copy(s_bf[:, a : a + 1], s_ps[a][:])
        # dotw[p, a, g, m] = s_a[p] * (g == m)
        for a in range(A):
            nc.vector.tensor_mul(
                dotw[:, a], eye_sb[:],
                s_bf[:, a : a + 1].unsqueeze(2).broadcast_to([128, NP, NP]),
            )

        # ---- node chunks: square, then PE reduces (sq then dots)
        for ci, (g0, w) in enumerate(chunks):
            c0, c1 = g0 * G, (g0 + w) * G
            if SQ_ENG[ci] == "S":
                nc.scalar.activation(
                    vsq[:, :, c0:c1], vt_sb[:, :, c0:c1],
                    mybir.ActivationFunctionType.Square,
                )
            else:
                nc.vector.tensor_mul(
                    vsq[:, :, c0:c1], vt_sb[:, :, c0:c1], vt_sb[:, :, c0:c1]
                )
            for g in range(g0, g0 + w):
                p, r = (0, g) if g < NP else (1, g - NP)
                first = g in (0, NP)
                last = g in (NP - 1, NG - 1)
                for a in range(A):
                    nc.tensor.matmul(
                        sq_ps[p][:],
                        eye_sb[:, r, :],
                        vsq[:, a, g * G : (g + 1) * G],
                        start=(first and a == 0),
                        stop=(last and a == 1),
                    )
            for g in range(g0, g0 + w):
                p, r = (0, g) if g < NP else (1, g - NP)
                first = g in (0, NP)
                last = g in (NP - 1, NG - 1)
                for a in range(A):
                    nc.tensor.matmul(
                        dot_ps[p][:],
                        dotw[:, a, r, :],
                        vt_sb[:, a, g * G : (g + 1) * G],
                        start=(first and a == 0),
                        stop=(last and a == 1),
                    )
            # pair tails as soon as a pair's groups are complete
            if g0 + w in (NP, NG):
                p = 0 if g0 + w == NP else 1
                r0 = 0 if p == 0 else NP
                nc.scalar.sqrt(vn[p][:], sq_ps[p][:])
                if TAIL_DIVIDE:
                    nc.vector.tensor_tensor(
                        res[p][:], dot_ps[p][:], vn[p][:],
                        op=mybir.AluOpType.divide,
                    )
                else:
                    nc.vector.reciprocal(isv[p][:], vn[p][:])
                    nc.vector.tensor_mul(res[p][:], dot_ps[p][:], isv[p][:])
                nc.sync.dma_start(out_v[r0 : r0 + NP, :], res[p][:])

    nc.compile()
    return nc


def _get_nc():
    if "nc" not in _cache:
        _cache["nc"] = _build()
    return _cache["nc"]


def _eye():
    if "eye" not in _cache:
        NP = PAIR_SPLIT
        e = np.zeros((128, NP, NP), dtype=BF)
        for g in range(NP):
            e[:, g, g] = 1.0
        _cache["eye"] = np.ascontiguousarray(e.reshape(128, NP * NP))
    return _cache["eye"]


def run(pred, target, node_emb, trace=False, **trace_kwargs):
    """Returns (full_output [50000] f32, BassKernelResults)."""
    target = np.asarray(target, dtype=np.float32)
    node_emb = np.asarray(node_emb, dtype=np.float32)
    tgt_bf = np.ascontiguousarray(target).astype(BF)

    nc = _get_nc()
    in_maps = []
    for c in range(N_CORES):
        shard = np.empty((NPAD, D), dtype=np.float32)
        shard[:NPC] = node_emb[c * NPC : (c + 1) * NPC]
        shard[NPC:] = node_emb[: NPAD - NPC]  # pad with real rows (no 0-norm)
        vtp = shard.T.astype(BF, order="C")   # [256, 6272] bf16, d-major
        in_maps.append({"target": tgt_bf, "vt": vtp, "eye": _eye()})

    res = bass_utils.run_bass_kernel_spmd(
        nc, in_maps, list(range(N_CORES)), trace=trace, **trace_kwargs
    )
    parts = [res.results[c]["out"][:NPC] for c in range(N_CORES)]
    return np.concatenate(parts).astype(np.float32), res


def kernel(pred, target, node_emb):
    out, _ = run(pred, target, node_emb)
    return out
